# revision 1
# baseline (speedup 1.0000x reference)
"""GroupSupConLoss on 8 Trainium2 NeuronCores.

loss = mean over anchors i of (logsumexp_{j!=i}(sim[i,j]) - mean_{j pos}(sim[i,j]))
with sim = E @ E.T / tau.

Device does the O(B^2 D) part: each core owns 1024 rows of the similarity
matrix and computes Z[i] = sum_{j != i} exp(sim[i,j]) via a bf16 GEMM with a
fused exp+row-sum epilogue on the scalar engine (accum_out).

Host does the O(B D) part: positives via the group-sum identity
  sum_pos[i] = (<e_i, G[label_i]> - <e_i, e_i>) / tau,  G[c] = sum of e_j with label c
plus counts, logs, and the final anchor mean (float64).

Sharding trick: each core receives E^T with columns ROTATED so that its own
1024-row block sits at columns 0..1023. The column-sum Z is permutation
invariant, and the diagonal block then sits at a core-independent position,
so one identical SPMD program runs on all 8 cores. The diagonal is masked on
the tensor engine itself: one extra accumulation matmul per diagonal bank
(identity stationary operand x a -1e30 diagonal-block moving operand), so
exp() flushes those elements to 0. No collectives; host sums the 8 partial
outputs.

Structure per core (modeled 229.5 us/core vs 218.6 us pure-PE floor at bf16,
i.e. ~1.05x the 78.6 TF/s roofline):
  - W = resident [128, 8k, 1024] block (cols 0..1023): stationary matmul
    operand for every tile AND the moving operand for region 0. Loaded as 8
    per-k DMAs so the PE starts on the first k-chunk ~2 us in.
  - streamed column groups (one 3D DMA each, double-buffered); the last
    1024-col group is split 512+512 to shorten the final-ACT tail.
  - Per (region, row-tile): PSUM [128, cols] accumulated over 8 k-chunks per
    512-col bank, then one ScalarE exp (scale=1/tau) with accum_out writing
    the row-sum directly; per-region partial sums reduced at the end.
"""

import numpy as np
import ml_dtypes

import concourse.bacc as bacc
import concourse.mybir as mybir
from concourse.tile import TileContext

B = 8192           # batch
D = 1024           # embed dim
NCORES = 8
RPC = B // NCORES  # rows per core = 1024
NK = D // 128      # 8 contraction chunks
NRT = RPC // 128   # 8 row tiles per core
WCOLS = 1024       # resident region (must equal RPC: holds the diagonal)
GCOLS = 1024       # streamed group width
NGRP = (B - WCOLS) // GCOLS
NREG = 1 + NGRP + 1  # last 1024-col group is split into two 512s
TAU = 0.1
NEG_BIG = -1.0e30

_NC_CACHE = {}


def _build_nc(reps: int = 1):
    nc = bacc.Bacc(None, target_bir_lowering=False)
    etrot = nc.declare_dram_parameter(
        "etrot", [D, B], mybir.dt.bfloat16, isOutput=False
    )
    ident = nc.declare_dram_parameter(
        "ident", [128, 128], mybir.dt.bfloat16, isOutput=False
    )
    negi = nc.declare_dram_parameter(
        "negi", [128, 128], mybir.dt.bfloat16, isOutput=False
    )
    zout = nc.declare_dram_parameter(
        "zout", [128, NRT], mybir.dt.float32, isOutput=True
    )
    et3 = etrot.rearrange("(nk p) c -> p nk c", p=128)

    with TileContext(nc) as tc:
        with (
            tc.tile_pool(name="singles", bufs=1) as singles,
            tc.tile_pool(name="rhsp", bufs=2) as rhsp,
            tc.tile_pool(name="psump", bufs=2, space="PSUM") as psump,
            tc.tile_pool(name="expp", bufs=2) as expp,
        ):
            W = singles.tile([128, NK, WCOLS], mybir.dt.bfloat16, name="W")
            # Per-k transfers: the first k-chunk lands in ~2 us so the PE
            # starts almost immediately; later chunks stream in behind it.
            for k in range(NK):
                nc.sync.dma_start(
                    out=W[:, k : k + 1, :], in_=et3[:, k : k + 1, 0:WCOLS]
                )
            ident_sb = singles.tile([128, 128], mybir.dt.bfloat16, name="ident_sb")
            nc.sync.dma_start(out=ident_sb, in_=ident[:, :])
            negi_sb = singles.tile([128, 128], mybir.dt.bfloat16, name="negi_sb")
            nc.sync.dma_start(out=negi_sb, in_=negi[:, :])
            acc = singles.tile([128, NRT, NREG], mybir.dt.float32, name="acc")
            zt = singles.tile([128, NRT], mybir.dt.float32, name="zt")

            regions = [("W", 0, WCOLS)] + [
                ("G", WCOLS + i * GCOLS, GCOLS) for i in range(NGRP - 1)
            ] + [("G", B - GCOLS, GCOLS // 2), ("G", B - GCOLS // 2, GCOLS // 2)]
            for rep in range(reps):
                for ri, (kind, col0, cols) in enumerate(regions):
                    if kind == "W":
                        rhs3 = W
                        rcol0 = 0
                    else:
                        rhs3 = rhsp.tile(
                            [128, NK, cols],
                            mybir.dt.bfloat16,
                            name=f"rhs_{rep}_{ri}",
                            tag="rhs",
                        )
                        nc.sync.dma_start(
                            out=rhs3[:, :, :], in_=et3[:, :, col0 : col0 + cols]
                        )
                        rcol0 = col0
                    nsub = cols // 512
                    for rt in range(NRT):
                        ps = psump.tile(
                            [128, cols],
                            mybir.dt.float32,
                            name=f"ps_{rep}_{ri}_{rt}",
                            tag="ps",
                        )
                        for sub in range(nsub):
                            # Diagonal: rotated column rt*128+p is the global
                            # row of partition p; always inside the W region.
                            diag_here = kind == "W" and sub == rt // 4
                            for k in range(NK):
                                nc.tensor.matmul(
                                    ps[:, sub * 512 : (sub + 1) * 512],
                                    W[:, k, rt * 128 : (rt + 1) * 128],
                                    rhs3[:, k, sub * 512 : (sub + 1) * 512],
                                    start=(k == 0),
                                    stop=(k == NK - 1) and not diag_here,
                                )
                            if diag_here:
                                # N=128 accumulation matmul adds -1e30 exactly
                                # on the diagonal positions of this row tile.
                                nc.tensor.matmul(
                                    ps[:, rt * 128 : (rt + 1) * 128],
                                    ident_sb,
                                    negi_sb,
                                    start=False,
                                    stop=True,
                                )
                        ex = expp.tile(
                            [128, cols],
                            mybir.dt.bfloat16,
                            name=f"ex_{rep}_{ri}_{rt}",
                            tag="ex",
                        )
                        nc.scalar.activation(
                            out=ex,
                            in_=ps,
                            func=mybir.ActivationFunctionType.Exp,
                            scale=1.0 / TAU,
                            accum_out=acc[:, rt, ri : ri + 1],
                        )
                        if ri == NREG - 1:
                            nc.vector.reduce_sum(
                                zt[:, rt : rt + 1],
                                acc[:, rt, :],
                                axis=mybir.AxisListType.X,
                            )
            nc.sync.dma_start(out=zout[:, :], in_=zt)
    nc.finalize()
    return nc


def _get_nc():
    if "nc" not in _NC_CACHE:
        _NC_CACHE["nc"] = _build_nc()
    return _NC_CACHE["nc"]


def _make_runner(nc=None, key="runner"):
    """Build a cached jitted SPMD executor for the bass program (mirrors
    concourse.bass2jax.run_bass_via_pjrt, but reusable across calls without
    retracing)."""
    if key in _NC_CACHE:
        return _NC_CACHE[key]

    import jax
    import concourse.mybir as mybir_
    from concourse import bass2jax
    from concourse.bass2jax import _bass_exec_p, partition_id_tensor
    from jax.sharding import Mesh, PartitionSpec
    from jax.experimental.shard_map import shard_map

    if nc is None:
        nc = _get_nc()
    bass2jax.install_neuronx_cc_hook()

    partition_name = nc.partition_id_tensor.name if nc.partition_id_tensor else None
    in_names, out_names, out_avals, zero_outs = [], [], [], []
    for alloc in nc.m.functions[0].allocations:
        if not isinstance(alloc, mybir_.MemoryLocationSet):
            continue
        name = alloc.memorylocations[0].name
        if alloc.kind == "ExternalInput":
            if name != partition_name:
                in_names.append(name)
        elif alloc.kind == "ExternalOutput":
            shape = tuple(alloc.tensor_shape)
            dtype = mybir_.dt.np(alloc.dtype)
            out_names.append(name)
            out_avals.append(jax.core.ShapedArray(shape, dtype))
            zero_outs.append(np.zeros(shape, dtype))
    n_params = len(in_names)
    all_in_names = list(in_names) + list(out_names)
    if partition_name is not None:
        all_in_names.append(partition_name)
    donate = tuple(range(n_params, n_params + len(out_avals)))

    def _body(*args):
        operands = list(args)
        if partition_name is not None:
            operands.append(partition_id_tensor())
        outs = _bass_exec_p.bind(
            *operands,
            out_avals=tuple(out_avals),
            in_names=tuple(all_in_names),
            out_names=tuple(out_names),
            lowering_input_output_aliases=(),
            sim_require_finite=True,
            sim_require_nnan=True,
            nc=nc,
        )
        return tuple(outs)

    devices = jax.devices()[:NCORES]
    mesh = Mesh(np.asarray(devices), ("core",))
    spec = PartitionSpec("core")
    sharded = jax.jit(
        shard_map(
            _body,
            mesh=mesh,
            in_specs=(spec,) * (n_params + len(out_avals)),
            out_specs=(spec,) * len(out_names),
            check_rep=False,
        ),
        donate_argnums=donate,
        keep_unused=True,
    )

    def run(in_maps, staged=None):
        """in_maps: list of per-core dicts. staged: optional pre-staged device
        arrays for the concatenated params (skips H2D)."""
        if staged is None:
            concat_in = [
                np.concatenate([np.asarray(m[name]) for m in in_maps], axis=0)
                for name in in_names
            ]
        else:
            concat_in = staged
        concat_zeros = [
            np.zeros((NCORES * z.shape[0], *z.shape[1:]), z.dtype) for z in zero_outs
        ]
        out_arrs = sharded(*concat_in, *concat_zeros)
        return [
            {
                name: np.asarray(out_arrs[i]).reshape(NCORES, *out_avals[i].shape)[c]
                for i, name in enumerate(out_names)
            }
            for c in range(NCORES)
        ]

    run.in_names = in_names
    run.mesh = mesh
    run.spec = spec
    run.sharded = sharded
    run.zero_outs = zero_outs
    _NC_CACHE[key] = run
    return run


def _make_in_maps(embeddings_f32: np.ndarray):
    et = np.ascontiguousarray(embeddings_f32.T).astype(ml_dtypes.bfloat16)  # [D, B]
    ident = np.eye(128, dtype=ml_dtypes.bfloat16)
    negi = (NEG_BIG * np.eye(128, dtype=np.float32)).astype(ml_dtypes.bfloat16)

    in_maps = []
    for c in range(NCORES):
        etrot = np.roll(et, -c * RPC, axis=1)
        in_maps.append(
            {"etrot": np.ascontiguousarray(etrot), "ident": ident, "negi": negi}
        )
    return in_maps


def _device_Z(embeddings_f32: np.ndarray):
    """Run the 8-core kernel; returns Z[B] = row sums of exp(sim), diag
    excluded."""
    run = _make_runner()
    results = run(_make_in_maps(embeddings_f32))
    Z = np.concatenate(
        [np.asarray(results[c]["zout"]).T.reshape(-1) for c in range(NCORES)]
    )
    return Z


def kernel(embeddings: np.ndarray, labels: np.ndarray) -> np.ndarray:
    E = np.asarray(embeddings, dtype=np.float32)
    labels = np.asarray(labels)

    Z = _device_Z(E)

    # Host epilogue in float64 (O(B*D) work).
    Ef = E.astype(np.float64)
    lse = np.log(Z.astype(np.float64))

    nclass = int(labels.max()) + 1
    counts = np.bincount(labels, minlength=nclass)
    num_pos = counts[labels] - 1
    G = np.zeros((nclass, D), dtype=np.float64)
    np.add.at(G, labels, Ef)
    sum_pos = (
        np.einsum("ij,ij->i", Ef, G[labels]) - np.einsum("ij,ij->i", Ef, Ef)
    ) / TAU
    mean_pos = sum_pos / np.maximum(num_pos, 1)
    has_pos = num_pos > 0
    loss_i = lse - mean_pos
    loss = np.sum(np.where(has_pos, loss_i, 0.0)) / max(int(has_pos.sum()), 1)
    return np.float32(loss)



# revision 2
# speedup vs baseline: 2.2993x; 2.2993x over previous
"""GroupSupConLoss on 8 Trainium2 NeuronCores.

loss = mean over anchors i of (logsumexp_{j!=i}(sim[i,j]) - mean_{j pos}(sim[i,j]))
with sim = E @ E.T / tau.

Device does the O(B^2 D) part: each core owns 1024 rows of the similarity
matrix and computes Z[i] = sum_{j != i} exp(sim[i,j]) via an fp8e4m3
DoubleRow GEMM (2x PE throughput vs bf16) with a fused exp+row-sum epilogue
on the scalar engine (accum_out). Embeddings are scaled by S=512 and
quantized to fp8; the activation scale 1/(S^2 tau) undoes it. DoubleRow
packs k-chunk pairs: lhsT/rhs APs are [128, 2, cols] slices of the resident
[128, 8, 8192] ET tile (contraction = 256 per matmul).

Host does the O(B D) part: positives via the group-sum identity
  sum_pos[i] = (<e_i, G[label_i]> - <e_i, e_i>) / tau,  G[c] = sum of e_j with label c
plus counts, logs, and the final anchor mean (float64).

Sharding trick: each core receives E^T with columns ROTATED so that its own
1024-row block sits at columns 0..1023. The column-sum Z is permutation
invariant, and the diagonal block then sits at a core-independent position,
so one identical SPMD program runs on all 8 cores. The diagonal is masked on
the tensor engine itself: one extra accumulation matmul per diagonal bank
(identity stationary operand x a -1e30 diagonal-block moving operand, both
bf16 - mixed dtype with the fp8 group is fine since PSUM accumulates fp32),
so exp() flushes those elements to 0. No collectives; host sums the 8
partial outputs.

Structure per core:
  - ET = resident [128, 8, 8192] fp8 tile (64KB/partition), loaded as 4
    column-block DMAs so the PE starts on the first 2048 cols ~6 us in.
  - group-major loop: for each 2048-col group, for each of 8 row tiles:
    PSUM [128, 2048] (4 banks, double buffered) accumulated with 16 fp8
    DoubleRow matmuls (4 subtiles x 4 k-pairs), then one ScalarE exp
    (scale=1/(S^2 tau)) over all 2048 cols with accum_out row-sums.
    Group-major order means the first DMA unblocks 8 row tiles of work.
"""

import numpy as np
import ml_dtypes

import concourse.bacc as bacc
import concourse.mybir as mybir
from concourse.tile import TileContext

B = 8192           # batch
D = 1024           # embed dim
NCORES = 8
RPC = B // NCORES  # rows per core = 1024
NK = D // 128      # 8 contraction chunks
NKP = NK // 2      # 4 DoubleRow k-pairs
NRT = RPC // 128   # 8 row tiles per core
GCOLS = 2048       # activation group width (4 PSUM banks)
NG = B // GCOLS    # 4 groups
TAU = 0.1
S = 512.0          # fp8 pre-scale
NEG_BIG = -1.0e30

_NC_CACHE = {}


def _build_nc(reps: int = 1):
    nc = bacc.Bacc(None, target_bir_lowering=False)
    etrot = nc.declare_dram_parameter(
        "etrot", [D, B], mybir.dt.float8e4, isOutput=False
    )
    ident = nc.declare_dram_parameter(
        "ident", [128, 128], mybir.dt.bfloat16, isOutput=False
    )
    negi = nc.declare_dram_parameter(
        "negi", [128, 128], mybir.dt.bfloat16, isOutput=False
    )
    zout = nc.declare_dram_parameter(
        "zout", [128, NRT], mybir.dt.float32, isOutput=True
    )
    et3 = etrot.rearrange("(nk p) c -> p nk c", p=128)

    DR = mybir.MatmulPerfMode.DoubleRow

    with TileContext(nc) as tc:
        with (
            tc.tile_pool(name="singles", bufs=1) as singles,
            tc.tile_pool(name="psump", bufs=2, space="PSUM") as psump,
            tc.tile_pool(name="expp", bufs=2) as expp,
        ):
            ET = singles.tile([128, NK, B], mybir.dt.float8e4, name="ET")
            # Column-block transfers: the first 2048 cols land first so the
            # PE starts while the rest stream in behind it.
            for g in range(NG):
                nc.sync.dma_start(
                    out=ET[:, :, g * GCOLS : (g + 1) * GCOLS],
                    in_=et3[:, :, g * GCOLS : (g + 1) * GCOLS],
                )
            ident_sb = singles.tile([128, 128], mybir.dt.bfloat16, name="ident_sb")
            nc.sync.dma_start(out=ident_sb, in_=ident[:, :])
            negi_sb = singles.tile([128, 128], mybir.dt.bfloat16, name="negi_sb")
            nc.sync.dma_start(out=negi_sb, in_=negi[:, :])
            acc = singles.tile([128, NRT, NG], mybir.dt.float32, name="acc")
            zt = singles.tile([128, NRT], mybir.dt.float32, name="zt")

            for rep in range(reps):
                for g in range(NG):
                    for rt in range(NRT):
                        ps = psump.tile(
                            [128, GCOLS],
                            mybir.dt.float32,
                            name=f"ps_{rep}_{g}_{rt}",
                            tag="ps",
                        )
                        for sub in range(GCOLS // 512):
                            col0 = g * GCOLS + sub * 512
                            # Rotated diagonal block: global row of partition p
                            # within row tile rt is rotated column rt*128+p,
                            # always inside group 0.
                            diag_here = g == 0 and sub == rt // 4
                            for kp in range(NKP):
                                nc.tensor.matmul(
                                    ps[:, sub * 512 : (sub + 1) * 512],
                                    ET[:, 2 * kp : 2 * kp + 2, rt * 128 : (rt + 1) * 128],
                                    ET[:, 2 * kp : 2 * kp + 2, col0 : col0 + 512],
                                    start=(kp == 0),
                                    stop=(kp == NKP - 1) and not diag_here,
                                    perf_mode=DR,
                                )
                            if diag_here:
                                # N=128 accumulation matmul adds -1e30 exactly
                                # on the diagonal positions of this row tile.
                                nc.tensor.matmul(
                                    ps[:, rt * 128 : (rt + 1) * 128],
                                    ident_sb,
                                    negi_sb,
                                    start=False,
                                    stop=True,
                                )
                        ex = expp.tile(
                            [128, GCOLS],
                            mybir.dt.bfloat16,
                            name=f"ex_{rep}_{g}_{rt}",
                            tag="ex",
                        )
                        nc.scalar.activation(
                            out=ex,
                            in_=ps,
                            func=mybir.ActivationFunctionType.Exp,
                            scale=1.0 / (S * S * TAU),
                            accum_out=acc[:, rt, g : g + 1],
                        )
                        if g == NG - 1:
                            nc.vector.reduce_sum(
                                zt[:, rt : rt + 1],
                                acc[:, rt, :],
                                axis=mybir.AxisListType.X,
                            )
            nc.sync.dma_start(out=zout[:, :], in_=zt)
    nc.finalize()
    return nc


def _get_nc():
    if "nc" not in _NC_CACHE:
        _NC_CACHE["nc"] = _build_nc()
    return _NC_CACHE["nc"]


def _make_runner(nc=None, key="runner"):
    """Build a cached jitted SPMD executor for the bass program (mirrors
    concourse.bass2jax.run_bass_via_pjrt, but reusable across calls without
    retracing)."""
    if key in _NC_CACHE:
        return _NC_CACHE[key]

    import jax
    import concourse.mybir as mybir_
    from concourse import bass2jax
    from concourse.bass2jax import _bass_exec_p, partition_id_tensor
    from jax.sharding import Mesh, PartitionSpec
    from jax.experimental.shard_map import shard_map

    if nc is None:
        nc = _get_nc()
    bass2jax.install_neuronx_cc_hook()

    partition_name = nc.partition_id_tensor.name if nc.partition_id_tensor else None
    in_names, out_names, out_avals, zero_outs = [], [], [], []
    for alloc in nc.m.functions[0].allocations:
        if not isinstance(alloc, mybir_.MemoryLocationSet):
            continue
        name = alloc.memorylocations[0].name
        if alloc.kind == "ExternalInput":
            if name != partition_name:
                in_names.append(name)
        elif alloc.kind == "ExternalOutput":
            shape = tuple(alloc.tensor_shape)
            dtype = mybir_.dt.np(alloc.dtype)
            out_names.append(name)
            out_avals.append(jax.core.ShapedArray(shape, dtype))
            zero_outs.append(np.zeros(shape, dtype))
    n_params = len(in_names)
    all_in_names = list(in_names) + list(out_names)
    if partition_name is not None:
        all_in_names.append(partition_name)
    donate = tuple(range(n_params, n_params + len(out_avals)))

    def _body(*args):
        operands = list(args)
        if partition_name is not None:
            operands.append(partition_id_tensor())
        outs = _bass_exec_p.bind(
            *operands,
            out_avals=tuple(out_avals),
            in_names=tuple(all_in_names),
            out_names=tuple(out_names),
            lowering_input_output_aliases=(),
            sim_require_finite=True,
            sim_require_nnan=True,
            nc=nc,
        )
        return tuple(outs)

    devices = jax.devices()[:NCORES]
    mesh = Mesh(np.asarray(devices), ("core",))
    spec = PartitionSpec("core")
    sharded = jax.jit(
        shard_map(
            _body,
            mesh=mesh,
            in_specs=(spec,) * (n_params + len(out_avals)),
            out_specs=(spec,) * len(out_names),
            check_rep=False,
        ),
        donate_argnums=donate,
        keep_unused=True,
    )

    def run(in_maps, staged=None):
        """in_maps: list of per-core dicts. staged: optional pre-staged device
        arrays for the concatenated params (skips H2D)."""
        if staged is None:
            concat_in = [
                np.concatenate([np.asarray(m[name]) for m in in_maps], axis=0)
                for name in in_names
            ]
        else:
            concat_in = staged
        concat_zeros = [
            np.zeros((NCORES * z.shape[0], *z.shape[1:]), z.dtype) for z in zero_outs
        ]
        out_arrs = sharded(*concat_in, *concat_zeros)
        return [
            {
                name: np.asarray(out_arrs[i]).reshape(NCORES, *out_avals[i].shape)[c]
                for i, name in enumerate(out_names)
            }
            for c in range(NCORES)
        ]

    run.in_names = in_names
    run.mesh = mesh
    run.spec = spec
    run.sharded = sharded
    run.zero_outs = zero_outs
    _NC_CACHE[key] = run
    return run


def _make_in_maps(embeddings_f32: np.ndarray):
    et = np.ascontiguousarray(embeddings_f32.T * S).astype(
        ml_dtypes.float8_e4m3
    )  # [D, B]
    ident = np.eye(128, dtype=ml_dtypes.bfloat16)
    negi = (NEG_BIG * np.eye(128, dtype=np.float32)).astype(ml_dtypes.bfloat16)

    in_maps = []
    for c in range(NCORES):
        etrot = np.roll(et, -c * RPC, axis=1)
        in_maps.append(
            {"etrot": np.ascontiguousarray(etrot), "ident": ident, "negi": negi}
        )
    return in_maps


def _device_Z(embeddings_f32: np.ndarray):
    """Run the 8-core kernel; returns Z[B] = row sums of exp(sim), diag
    excluded."""
    run = _make_runner()
    results = run(_make_in_maps(embeddings_f32))
    Z = np.concatenate(
        [np.asarray(results[c]["zout"]).T.reshape(-1) for c in range(NCORES)]
    )
    return Z


def kernel(embeddings: np.ndarray, labels: np.ndarray) -> np.ndarray:
    E = np.asarray(embeddings, dtype=np.float32)
    labels = np.asarray(labels)

    Z = _device_Z(E)

    # Host epilogue in float64 (O(B*D) work).
    Ef = E.astype(np.float64)
    lse = np.log(Z.astype(np.float64))

    nclass = int(labels.max()) + 1
    counts = np.bincount(labels, minlength=nclass)
    num_pos = counts[labels] - 1
    G = np.zeros((nclass, D), dtype=np.float64)
    np.add.at(G, labels, Ef)
    sum_pos = (
        np.einsum("ij,ij->i", Ef, G[labels]) - np.einsum("ij,ij->i", Ef, Ef)
    ) / TAU
    mean_pos = sum_pos / np.maximum(num_pos, 1)
    has_pos = num_pos > 0
    loss_i = lse - mean_pos
    loss = np.sum(np.where(has_pos, loss_i, 0.0)) / max(int(has_pos.sum()), 1)
    return np.float32(loss)


# revision 7
# speedup vs baseline: 3.1562x; 1.3727x over previous
"""GroupSupConLoss on 8 Trainium2 NeuronCores.

loss = mean over anchors i of (logsumexp_{j!=i}(sim[i,j]) - mean_{j pos}(sim[i,j]))
with sim = E @ E.T / tau.

Device computes Z[i] = sum_{j != i} exp(sim[i,j]) exploiting the SYMMETRY of
sim: each unordered pair {i, j} is computed once. Core c owns 8 row blocks of
128 (1024 rows); for each row block I it computes the band of column blocks
J = I .. I+31 (mod 64, in rotated space) with an fp8e4m3 DoubleRow GEMM
(2x PE throughput), exps it on the scalar engine (accum_out gives the row
sums), and writes the exp'd tiles to SBUF in fp8. The mirrored contributions
(rows of block J get column sums of tile (I,J)) are then produced by cheap
fp8 DoubleRow matmuls with an all-ones stationary vector, pairing two row
blocks per matmul. The antipodal block pair {I, I+32} is computed TRANSPOSED
(stationary = partner block), so its column sums are this core's row sums -
both owners compute it, which keeps the program SPMD-uniform.

Host does the O(B D) part: positives via the group-sum identity
  sum_pos[i] = (<e_i, G[label_i]> - <e_i, e_i>) / tau,  G[c] = sum of e_j with label c
plus counts, logs, scattering the device's row/col partial sums, and the
final anchor mean (float64).

Sharding trick: each core receives E^T with columns ROTATED so that its own
1024-row block sits at columns 0..1023; it only needs rotated columns
0..5119. Row block i's band is rotated columns [i*128, i*128+4096); the
diagonal then always sits at the first 128 columns of the band, masked on
the tensor engine by one extra accumulation matmul (identity x -1e30
diagonal, bf16 - mixed dtype with the fp8 group is fine since PSUM
accumulates fp32). No collectives; host routes the 8 cores' partial sums.

Per core:
  - ET = resident [128, 8, 5120] fp8 tile; 512-col DMA blocks first so the
    PE starts ~7 us in, then 1024-col blocks.
  - main loop: 8 row tiles x 2 groups of 2048 cols: PSUM [128, 2048]
    (4 banks, double buffered) accumulated with 16 fp8 DoubleRow matmuls,
    then one ScalarE exp (scale=1/(S^2 tau), fp8 out to ex_abs, accum_out
    row-sums). ex_abs stores tiles at ABSOLUTE rotated-column offsets so the
    later column-sum matmuls can pair row blocks i, i+1 with a uniform
    stride.
  - o32 phase: 8 transposed 128x128 tiles vs the antipodal block, one exp,
    8 column-sum matmuls -> zo32 (this core's rows' sums over that block).
  - colsum phase: for rotated column block J=1..38, sum exp'd tiles over
    contributing row blocks i in [J-31, J-1] cap [0,7]: ceil(n/2) DoubleRow
    matmuls (ones x ex pair) accumulating into PSUM [1, 128*chunk], DMA'd
    out as zcol.
"""

import numpy as np
import ml_dtypes

import concourse.bacc as bacc
import concourse.mybir as mybir
from concourse.tile import TileContext

B = 8192           # batch
D = 1024           # embed dim
NCORES = 8
RPC = B // NCORES  # rows per core = 1024
NK = D // 128      # 8 contraction chunks
NKP = NK // 2      # 4 DoubleRow k-pairs
NRT = RPC // 128   # 8 row tiles per core
NBAND = 32         # main band width in 128-col blocks (offsets 0..31)
BANDC = NBAND * 128          # 4096
ETC = BANDC + RPC            # 5120 rotated cols resident
GCOLS = 2048       # activation group width (4 PSUM banks)
NGG = BANDC // GCOLS         # 2 groups per row tile
NJ = NRT + NBAND - 1         # 38 col blocks needing colsums (J = 1..38)
TAU = 0.1
S = 512.0          # fp8 pre-scale
ASCALE = 1.0 / (S * S * TAU)
NEG_BIG = -1.0e30

_NC_CACHE = {}


def _build_nc(reps: int = 1):
    nc = bacc.Bacc(None, target_bir_lowering=False)
    etrot = nc.declare_dram_parameter(
        "etrot", [D, ETC], mybir.dt.float8e4, isOutput=False
    )
    ident = nc.declare_dram_parameter(
        "ident", [128, 128], mybir.dt.bfloat16, isOutput=False
    )
    negi = nc.declare_dram_parameter(
        "negi", [128, 128], mybir.dt.bfloat16, isOutput=False
    )
    zout = nc.declare_dram_parameter(
        "zout", [128, NRT], mybir.dt.float32, isOutput=True
    )
    zo32 = nc.declare_dram_parameter(
        "zo32", [1, RPC], mybir.dt.float32, isOutput=True
    )
    zcol = nc.declare_dram_parameter(
        "zcol", [1, NJ * 128], mybir.dt.float32, isOutput=True
    )
    et3 = etrot.rearrange("(nk p) c -> p nk c", p=128)

    DR = mybir.MatmulPerfMode.DoubleRow
    EXP = mybir.ActivationFunctionType.Exp

    with TileContext(nc) as tc:
        with (
            tc.tile_pool(name="singles", bufs=1) as singles,
            tc.tile_pool(name="psump", bufs=2, space="PSUM") as psump,
        ):
            ET = singles.tile([128, NK, ETC], mybir.dt.float8e4, name="ET")
            # Small blocks first so the first row tile's matmuls start early.
            dma_blocks = [(a, 512) for a in range(0, 2048, 512)] + [
                (a, 1024) for a in range(2048, ETC, 1024)
            ]
            for a, w in dma_blocks:
                nc.sync.dma_start(out=ET[:, :, a : a + w], in_=et3[:, :, a : a + w])
            ident_sb = singles.tile([128, 128], mybir.dt.bfloat16, name="ident_sb")
            nc.sync.dma_start(out=ident_sb, in_=ident[:, :])
            negi_sb = singles.tile([128, 128], mybir.dt.bfloat16, name="negi_sb")
            nc.sync.dma_start(out=negi_sb, in_=negi[:, :])
            # all-ones fp8 stationary for column sums; [128, 2, 16] so the
            # DoubleRow pair stride is 16 bytes (hw requires step % 16 == 0).
            ones_sb = singles.tile([128, 2, 16], mybir.dt.float8e4, name="ones_sb")
            nc.any.memset(ones_sb, 1.0)
            # exp'd tiles at absolute rotated-col offsets: row tile rt owns
            # ex_abs[:, rt, rt*128 : rt*128+4096)
            ex_abs = singles.tile([128, NRT, ETC], mybir.dt.float8e4, name="ex_abs")
            ex_o32 = singles.tile([128, RPC], mybir.dt.float8e4, name="ex_o32")
            acc = singles.tile([128, NRT, NGG], mybir.dt.float32, name="acc")
            zt = singles.tile([128, NRT], mybir.dt.float32, name="zt")
            # SBUF staging for PSUM-resident column sums (DMA can't read PSUM)
            zc32_sb = singles.tile([1, RPC], mybir.dt.float32, name="zc32_sb")
            zcol_sb = singles.tile([1, NJ * 128], mybir.dt.float32, name="zcol_sb")

            for rep in range(reps):
                # ---- main band: row tile rt x 2048-col groups ----
                for rt in range(NRT):
                    for gg in range(NGG):
                        ps = psump.tile(
                            [128, GCOLS],
                            mybir.dt.float32,
                            name=f"ps_{rep}_{rt}_{gg}",
                            tag="ps",
                        )
                        for sub in range(GCOLS // 512):
                            col0 = rt * 128 + gg * GCOLS + sub * 512
                            diag_here = gg == 0 and sub == 0
                            for kp in range(NKP):
                                nc.tensor.matmul(
                                    ps[:, sub * 512 : (sub + 1) * 512],
                                    ET[:, 2 * kp : 2 * kp + 2, rt * 128 : (rt + 1) * 128],
                                    ET[:, 2 * kp : 2 * kp + 2, col0 : col0 + 512],
                                    start=(kp == 0),
                                    stop=(kp == NKP - 1) and not diag_here,
                                    perf_mode=DR,
                                )
                            if diag_here:
                                # adds -1e30 exactly on the diagonal
                                nc.tensor.matmul(
                                    ps[:, 0:128],
                                    ident_sb,
                                    negi_sb,
                                    start=False,
                                    stop=True,
                                )
                        nc.scalar.activation(
                            out=ex_abs[
                                :, rt, rt * 128 + gg * GCOLS : rt * 128 + (gg + 1) * GCOLS
                            ],
                            in_=ps,
                            func=EXP,
                            scale=ASCALE,
                            accum_out=acc[:, rt, gg : gg + 1],
                        )
                        if gg == NGG - 1:
                            nc.vector.reduce_sum(
                                zt[:, rt : rt + 1],
                                acc[:, rt, :],
                                axis=mybir.AxisListType.X,
                            )

                # ---- o32: antipodal pair blocks, computed transposed ----
                ps32 = psump.tile(
                    [128, RPC], mybir.dt.float32, name=f"ps32_{rep}", tag="ps"
                )
                for rt in range(NRT):
                    for kp in range(NKP):
                        nc.tensor.matmul(
                            ps32[:, rt * 128 : (rt + 1) * 128],
                            ET[:, 2 * kp : 2 * kp + 2, BANDC + rt * 128 : BANDC + (rt + 1) * 128],
                            ET[:, 2 * kp : 2 * kp + 2, rt * 128 : (rt + 1) * 128],
                            start=(kp == 0),
                            stop=(kp == NKP - 1),
                            perf_mode=DR,
                        )
                nc.scalar.activation(out=ex_o32, in_=ps32, func=EXP, scale=ASCALE)
                pc32 = psump.tile(
                    [1, RPC], mybir.dt.float32, name=f"pc32_{rep}", tag="ps"
                )
                for rt in range(NRT):
                    # col sums of the transposed tile = row sums of my rows
                    nc.tensor.matmul(
                        pc32[:, rt * 128 : (rt + 1) * 128],
                        ones_sb[:, 0, 0:1],
                        ex_o32[:, rt * 128 : (rt + 1) * 128],
                        start=True,
                        stop=True,
                    )
                nc.vector.tensor_scalar_add(zc32_sb, pc32, 0.0)
                nc.sync.dma_start(out=zo32[:, :], in_=zc32_sb)

                # ---- colsum phase: mirrored contributions for J = 1..38 ----
                CHUNK = 16
                for j0 in range(1, NJ + 1, CHUNK):
                    jn = min(CHUNK, NJ + 1 - j0)
                    pc = psump.tile(
                        [1, jn * 128], mybir.dt.float32, name=f"pc_{rep}_{j0}", tag="ps"
                    )
                    for dj in range(jn):
                        J = j0 + dj
                        i0 = max(0, J - NBAND + 1)
                        i1 = min(NRT - 1, J - 1)  # inclusive
                        out_sl = pc[:, dj * 128 : (dj + 1) * 128]
                        i = i0
                        first = True
                        while i <= i1:
                            if i + 1 <= i1:
                                nc.tensor.matmul(
                                    out_sl,
                                    ones_sb[:, :, 0:1],
                                    ex_abs[:, i : i + 2, J * 128 : (J + 1) * 128],
                                    start=first,
                                    stop=(i + 2 > i1),
                                    perf_mode=DR,
                                )
                                i += 2
                            else:
                                nc.tensor.matmul(
                                    out_sl,
                                    ones_sb[:, 0, 0:1],
                                    ex_abs[:, i, J * 128 : (J + 1) * 128],
                                    start=first,
                                    stop=True,
                                )
                                i += 1
                            first = False
                    sl = zcol_sb[:, (j0 - 1) * 128 : (j0 - 1 + jn) * 128]
                    nc.vector.tensor_scalar_add(sl, pc, 0.0)
                    nc.sync.dma_start(
                        out=zcol[:, (j0 - 1) * 128 : (j0 - 1 + jn) * 128], in_=sl
                    )
            nc.sync.dma_start(out=zout[:, :], in_=zt)
    nc.finalize()
    return nc


def _get_nc():
    if "nc" not in _NC_CACHE:
        _NC_CACHE["nc"] = _build_nc()
    return _NC_CACHE["nc"]


def _make_runner(nc=None, key="runner"):
    """Build a cached jitted SPMD executor for the bass program (mirrors
    concourse.bass2jax.run_bass_via_pjrt, but reusable across calls without
    retracing)."""
    if key in _NC_CACHE:
        return _NC_CACHE[key]

    import jax
    import concourse.mybir as mybir_
    from concourse import bass2jax
    from concourse.bass2jax import _bass_exec_p, partition_id_tensor
    from jax.sharding import Mesh, PartitionSpec
    from jax.experimental.shard_map import shard_map

    if nc is None:
        nc = _get_nc()
    bass2jax.install_neuronx_cc_hook()

    partition_name = nc.partition_id_tensor.name if nc.partition_id_tensor else None
    in_names, out_names, out_avals, zero_outs = [], [], [], []
    for alloc in nc.m.functions[0].allocations:
        if not isinstance(alloc, mybir_.MemoryLocationSet):
            continue
        name = alloc.memorylocations[0].name
        if alloc.kind == "ExternalInput":
            if name != partition_name:
                in_names.append(name)
        elif alloc.kind == "ExternalOutput":
            shape = tuple(alloc.tensor_shape)
            dtype = mybir_.dt.np(alloc.dtype)
            out_names.append(name)
            out_avals.append(jax.core.ShapedArray(shape, dtype))
            zero_outs.append(np.zeros(shape, dtype))
    n_params = len(in_names)
    all_in_names = list(in_names) + list(out_names)
    if partition_name is not None:
        all_in_names.append(partition_name)
    donate = tuple(range(n_params, n_params + len(out_avals)))

    def _body(*args):
        operands = list(args)
        if partition_name is not None:
            operands.append(partition_id_tensor())
        outs = _bass_exec_p.bind(
            *operands,
            out_avals=tuple(out_avals),
            in_names=tuple(all_in_names),
            out_names=tuple(out_names),
            lowering_input_output_aliases=(),
            sim_require_finite=True,
            sim_require_nnan=True,
            nc=nc,
        )
        return tuple(outs)

    devices = jax.devices()[:NCORES]
    mesh = Mesh(np.asarray(devices), ("core",))
    spec = PartitionSpec("core")
    sharded = jax.jit(
        shard_map(
            _body,
            mesh=mesh,
            in_specs=(spec,) * (n_params + len(out_avals)),
            out_specs=(spec,) * len(out_names),
            check_rep=False,
        ),
        donate_argnums=donate,
        keep_unused=True,
    )

    def run(in_maps, staged=None):
        """in_maps: list of per-core dicts. staged: optional pre-staged device
        arrays for the concatenated params (skips H2D)."""
        if staged is None:
            concat_in = [
                np.concatenate([np.asarray(m[name]) for m in in_maps], axis=0)
                for name in in_names
            ]
        else:
            concat_in = staged
        concat_zeros = [
            np.zeros((NCORES * z.shape[0], *z.shape[1:]), z.dtype) for z in zero_outs
        ]
        out_arrs = sharded(*concat_in, *concat_zeros)
        return [
            {
                name: np.asarray(out_arrs[i]).reshape(NCORES, *out_avals[i].shape)[c]
                for i, name in enumerate(out_names)
            }
            for c in range(NCORES)
        ]

    run.in_names = in_names
    run.mesh = mesh
    run.spec = spec
    run.sharded = sharded
    run.zero_outs = zero_outs
    _NC_CACHE[key] = run
    return run


def _make_in_maps(embeddings_f32: np.ndarray):
    et = np.ascontiguousarray(embeddings_f32.T * S).astype(
        ml_dtypes.float8_e4m3
    )  # [D, B]
    ident = np.eye(128, dtype=ml_dtypes.bfloat16)
    negi = (NEG_BIG * np.eye(128, dtype=np.float32)).astype(ml_dtypes.bfloat16)

    in_maps = []
    for c in range(NCORES):
        etrot = np.roll(et, -c * RPC, axis=1)[:, :ETC]
        in_maps.append(
            {"etrot": np.ascontiguousarray(etrot), "ident": ident, "negi": negi}
        )
    return in_maps


def _device_Z(embeddings_f32: np.ndarray):
    """Run the 8-core kernel; returns Z[B] = row sums of exp(sim), diag
    excluded, assembled from row-band sums + mirrored column sums."""
    run = _make_runner()
    results = run(_make_in_maps(embeddings_f32))
    Z = np.zeros(B, dtype=np.float64)
    for c in range(NCORES):
        r = results[c]
        own = np.asarray(r["zout"]).T.reshape(-1).astype(np.float64)  # [1024]
        own += np.asarray(r["zo32"]).reshape(-1).astype(np.float64)
        Z[c * RPC : (c + 1) * RPC] += own
        col = np.asarray(r["zcol"]).reshape(-1).astype(np.float64)  # [NJ*128]
        tgt = (c * RPC + 128 + np.arange(NJ * 128)) % B
        np.add.at(Z, tgt, col)
    return Z


def kernel(embeddings: np.ndarray, labels: np.ndarray) -> np.ndarray:
    E = np.asarray(embeddings, dtype=np.float32)
    labels = np.asarray(labels)

    Z = _device_Z(E)

    # Host epilogue in float64 (O(B*D) work).
    Ef = E.astype(np.float64)
    lse = np.log(Z)

    nclass = int(labels.max()) + 1
    counts = np.bincount(labels, minlength=nclass)
    num_pos = counts[labels] - 1
    G = np.zeros((nclass, D), dtype=np.float64)
    np.add.at(G, labels, Ef)
    sum_pos = (
        np.einsum("ij,ij->i", Ef, G[labels]) - np.einsum("ij,ij->i", Ef, Ef)
    ) / TAU
    mean_pos = sum_pos / np.maximum(num_pos, 1)
    has_pos = num_pos > 0
    loss_i = lse - mean_pos
    loss = np.sum(np.where(has_pos, loss_i, 0.0)) / max(int(has_pos.sum()), 1)
    return np.float32(loss)
